# revision 46
# baseline (speedup 1.0000x reference)
"""GraphTransformer (2x GCNConv + global MHA) on 8 TRN2 NeuronCores.

Strategy (v2)
-------------
Nodes (N=4096) are sharded 512/core. The GCN scatter-add is a dense SpMM
against the normalized adjacency A (built on host from edge_index; pure index
preprocessing). All heavy matmuls that tolerate it run in fp8e4 with
DoubleRow double-pumping (2 MACs/cell/cycle):

  - conv1/conv2: lhsT = H (node-pair fp8 tiles [128,2,F]), rhs = A^T pair
    tiles [128,2,512]. A is pre-scaled x8, H1 x4, H2pre x16 on device; the
    scales are folded into the (host-scaled) weights / bias adds.
  - attn@V: lhsT = V' pairs [128,2,80h+65], rhs = exp(scores) fp8.

H1 = X@W1 is computed sharded (512 rows/core) and AllGathered in fp8 (2 MB),
as is H2pre. K-bias is dropped (softmax is invariant to per-query shifts).
conv relu+bias evacuations run on DVE (tensor_scalar add+max) with weight
scaling folded on host, keeping ACT free for the 64 softmax EXPs (the ~73 us
scalar-engine floor). Emission interleaves conv matmuls into the attention
j-loop so the PE fills the exp-stall gaps; softmax division uses a 2-step
Newton reciprocal on DVE from a constant seed.
"""

import os
import sys

import numpy as np
import ml_dtypes

try:
    import concourse  # noqa: F401
except ImportError:  # pragma: no cover
    sys.path.insert(0, "/opt/trn_rl_repo")

from concourse import bacc, bass, mybir, tile
from concourse.bass_utils import run_bass_kernel_spmd

P = 128
N_NODES = 4096
E_EDGES = 131072
IN_DIM = 256
HID = 512
CLS = 256
HEADS = 4
HDIM = 64
NC = 8
RPC = N_NODES // NC  # 512 rows per core

BF = mybir.dt.bfloat16
F32 = mybir.dt.float32
F32R = mybir.dt.float32r
F8 = mybir.dt.float8e4
AF = mybir.ActivationFunctionType
ALU = mybir.AluOpType
DR = mybir.MatmulPerfMode.DoubleRow

KCH_IN = IN_DIM // P    # 2
KCH_HID = HID // P      # 4
NT = N_NODES // P       # 32 node tiles
NPAIR = NT // 2         # 16 node-tile pairs
MT_Q = RPC // P         # 4 query tiles per core

# The adjacency factors exactly: A = dinv[dst] * cnt * dinv[src] with integer
# cnt (exact in fp8). dinv scalings ride per-partition evac scales + rank-1
# bias matmuls, so the conv matmuls see NO A-quantization error.
SC_H1 = 4.0      # H1' = 4*dinv*H1 in fp8
SC_H2 = 16.0     # hc = 16*dinv^2*H2pre in fp8
ATTN_FP8 = False  # es/v' in fp8 + DoubleRow attn@V (faster, more error)
SC_V = 8.0 if ATTN_FP8 else 1.0
ES_DT = mybir.dt.float8e4 if ATTN_FP8 else mybir.dt.bfloat16
# Newton seed for 1/D, D = SC_V * sum_k exp(s) ~ SC_V*4096
REC_SEED = 1.0 / (4096.0 * SC_V)

VSLOT = 80  # per-head slot in v' tiles (64 v dims + 1 ones + pad, 16B aligned)


def _emit(tc):
    nc = tc.nc

    # ---------------- I/O ----------------
    xT = nc.dram_tensor("xT", [IN_DIM, N_NODES], BF, kind="ExternalInput")
    xq = nc.dram_tensor("xq", [IN_DIM, RPC], BF, kind="ExternalInput")
    aTd = nc.dram_tensor("aTd", [P, NT, RPC], F8, kind="ExternalInput")
    w1 = nc.dram_tensor("w1", [IN_DIM, HID], BF, kind="ExternalInput")
    w2 = nc.dram_tensor("w2", [HID, HID], BF, kind="ExternalInput")
    lw = nc.dram_tensor("lw", [HID, CLS], BF, kind="ExternalInput")
    ipT = nc.dram_tensor("ipT", [IN_DIM, 3 * IN_DIM], BF, kind="ExternalInput")
    opw = nc.dram_tensor("opw", [IN_DIM, IN_DIM], BF, kind="ExternalInput")
    pw = nc.dram_tensor("pw", [IN_DIM, CLS], BF, kind="ExternalInput")
    bp = nc.dram_tensor("bp", [P, 14], F32, kind="ExternalInput")
    bpd = nc.dram_tensor("bpd", [P, NT], F32, kind="ExternalInput")
    bfrd = nc.dram_tensor("bfrd", [1, 3 * HID], BF, kind="ExternalInput")
    brow = nc.dram_tensor("brow", [1, CLS], BF, kind="ExternalInput")
    out = nc.dram_tensor("out", [RPC, CLS], F32, kind="ExternalOutput")

    from contextlib import ExitStack

    with ExitStack() as ctx:
        sb = ctx.enter_context(tc.tile_pool(name="sb", bufs=1))
        pp = ctx.enter_context(tc.tile_pool(name="pp", bufs=1, space="PSUM"))
        dr = ctx.enter_context(tc.tile_pool(name="dr", bufs=1, space="DRAM"))

        # ---------------- input DMA --------------------------------------
        # H1 needs w1 + both xT halves first; kT/v need ipT next. Split the
        # big early tensors across the two DMA queues; late-consumed tensors
        # (aTs, w2, opw, pw, lw) go after.
        xTs, ipTs = [], []
        for k in range(KCH_IN):
            t = sb.tile([P, N_NODES], BF, name=f"xTs{k}", tag=f"xTs{k}")
            xTs.append(t)
            t2 = sb.tile([P, 3 * IN_DIM], BF, name=f"ipTs{k}", tag=f"ipTs{k}")
            ipTs.append(t2)
        w1s = []
        for k in range(KCH_IN):
            t = sb.tile([P, HID], BF, name=f"w1s{k}", tag=f"w1s{k}")
            nc.gpsimd.dma_start(out=t, in_=w1[k * P:(k + 1) * P, :])
            w1s.append(t)
        nc.sync.dma_start(out=xTs[0], in_=xT[0:P, :])
        nc.gpsimd.dma_start(out=xTs[1], in_=xT[P:2 * P, :])
        nc.sync.dma_start(out=ipTs[0], in_=ipT[0:P, :])
        nc.gpsimd.dma_start(out=ipTs[1], in_=ipT[P:2 * P, :])
        bps = sb.tile([P, 14], F32, name="bps", tag="bps")
        nc.sync.dma_start(out=bps, in_=bp[:, :])
        bpds = sb.tile([P, NT], F32, name="bpds", tag="bpds")
        nc.sync.dma_start(out=bpds, in_=bpd[:, :])
        bfr = sb.tile([1, 3 * HID], BF, name="bfr", tag="bfr")
        nc.sync.dma_start(out=bfr, in_=bfrd[:, :])
        xqs = []
        for k in range(KCH_IN):
            t = sb.tile([P, RPC], BF, name=f"xqs{k}", tag=f"xqs{k}")
            nc.sync.dma_start(out=t, in_=xq[k * P:(k + 1) * P, :])
            xqs.append(t)
        w2s = []
        for k in range(KCH_HID):
            t = sb.tile([P, HID], BF, name=f"w2s{k}", tag=f"w2s{k}")
            nc.gpsimd.dma_start(out=t, in_=w2[k * P:(k + 1) * P, :])
            w2s.append(t)
        opws, pws = [], []
        for k in range(KCH_IN):
            t = sb.tile([P, IN_DIM], BF, name=f"opws{k}", tag=f"opws{k}")
            nc.gpsimd.dma_start(out=t, in_=opw[k * P:(k + 1) * P, :])
            opws.append(t)
            t2 = sb.tile([P, CLS], BF, name=f"pws{k}", tag=f"pws{k}")
            nc.gpsimd.dma_start(out=t2, in_=pw[k * P:(k + 1) * P, :])
            pws.append(t2)
        brows = sb.tile([1, CLS], BF, name="brows", tag="brows")
        nc.gpsimd.dma_start(out=brows, in_=brow[:, :])
        lws = []
        for k in range(KCH_HID):
            t = sb.tile([P, CLS], BF, name=f"lws{k}", tag=f"lws{k}")
            nc.gpsimd.dma_start(out=t, in_=lw[k * P:(k + 1) * P, :])
            lws.append(t)

        ones_bf = sb.tile([1, P], BF, name="ones_bf", tag="ones_bf")
        nc.vector.memset(ones_bf, 1.0)


        # per-partition scale/bias slices
        d4c = [bps[:, m:m + 1] for m in range(0, 4)]          # 4*dinv (m-tile)
        d16c = [bps[:, 4 + m:5 + m] for m in range(0, 4)]     # 16*dinv^2
        dfc = [bps[:, 8 + m:9 + m] for m in range(0, 4)]      # dinv
        bqc = [bps[:, 12 + m:13 + m] for m in range(0, 2)]    # bq

        # big late-consumed input: adjacency counts (conv1 needs it ~30us in)
        aTs = sb.tile([P, NT, RPC], F8, name="aTs", tag="aTs")
        nc.sync.dma_start(out=aTs, in_=aTd[:, :, :])
        # H1' = 4*dinv*(X @ W1) computed REPLICATED (no collective: the first
        # collective would absorb cross-core launch skew as dead PE time).
        # Node-pair fp8 tiles for the conv1 DoubleRow lhsT, produced in the
        # tl0 loop below (all 32 node tiles done by block n=3).
        H1f8 = [sb.tile([P, 2, HID], F8, name=f"H1f8_{j}", tag=f"H1f8_{j}")
                for j in range(NPAIR)]

        def emit_h1_tile(i):
            pt = pp.tile([P, HID], F32, name=f"h1p{i}", tag="mm", bufs=2)
            for k in range(KCH_IN):
                nc.tensor.matmul(out=pt, lhsT=xTs[k][:, i * P:(i + 1) * P],
                                 rhs=w1s[k], start=(k == 0),
                                 stop=(k == KCH_IN - 1))
            # alternate evacuation between DVE and ACT to balance engines
            if i % 2 == 0:
                nc.vector.tensor_scalar_mul(H1f8[i // 2][:, i % 2, :], pt,
                                            bpds[:, i:i + 1])
            else:
                nc.scalar.activation(H1f8[i // 2][:, i % 2, :], pt, AF.Copy,
                                     scale=bpds[:, i:i + 1])

        # ---------------- Phase B: qT (+bq) ---------------------------------
        qTs = []
        for m in range(2):
            pt = pp.tile([P, RPC], F32, name=f"q_ps{m}", tag="mm", bufs=2)
            for k in range(KCH_IN):
                nc.tensor.matmul(out=pt, lhsT=ipTs[k][:, m * P:(m + 1) * P],
                                 rhs=xqs[k], start=(k == 0), stop=(k == KCH_IN - 1))
            t = sb.tile([P, RPC], BF, name=f"qTs{m}", tag=f"qTs{m}")
            nc.vector.tensor_scalar_add(t, pt, bqc[m])
            qTs.append(t)

        # persistent attention state
        kTs = [sb.tile([P, N_NODES], BF, name=f"kTs{m}", tag=f"kTs{m}")
               for m in range(2)]
        vpd = [sb.tile([P, 2, HEADS * VSLOT], ES_DT, name=f"vpd{j}",
                       tag=f"vpd{j}")
               for j in range(NPAIR)]
        # ones columns for the softmax denominator (only the 8 columns the
        # attn@V lhsT slice actually reads; the pad columns stay untouched)
        for j in range(NPAIR):
            nc.gpsimd.memset(
                vpd[j].rearrange("p s (h c) -> p s h c", c=VSLOT)
                [:, :, :, HDIM:HDIM + 1], SC_V)
        oTs = [sb.tile([P, RPC], BF, name=f"oTs{m}", tag=f"oTs{m}")
               for m in range(2)]
        h1Ts, h2Ts = [], []

        def emit_kv_chunk(n):
            # kT columns n*512 .. +512 (both head-pair tiles), no k-bias
            # (softmax is invariant to per-query score shifts).
            for tl in range(2):
                pt = pp.tile([P, 512], F32, name=f"k_ps{tl}_{n}", tag="mm",
                             bufs=2)
                for k in range(KCH_IN):
                    nc.tensor.matmul(
                        out=pt,
                        lhsT=ipTs[k][:, IN_DIM + tl * P:IN_DIM + (tl + 1) * P],
                        rhs=xTs[k][:, n * 512:(n + 1) * 512],
                        start=(k == 0), stop=(k == KCH_IN - 1))
                nc.vector.tensor_copy(out=kTs[tl][:, n * 512:(n + 1) * 512],
                                      in_=pt)
            # v' for node tiles 4n..4n+3 (pairs 2n, 2n+1); the v bias is
            # folded into brow on the host (softmax weights sum to 1, so
            # bv rides through out_proj @ proj as a constant row).
            for i in range(4 * n, 4 * n + 4):
                pt = pp.tile([P, IN_DIM], F32, name=f"v_ps{i}", tag="mm",
                             bufs=2)
                for k in range(KCH_IN):
                    nc.tensor.matmul(out=pt, lhsT=xTs[k][:, i * P:(i + 1) * P],
                                     rhs=ipTs[k][:, 2 * IN_DIM:3 * IN_DIM],
                                     start=(k == 0), stop=(k == KCH_IN - 1))
                vv = (vpd[i // 2][:, i % 2, :]
                      .rearrange("p (h d) -> p h d", h=HEADS)[:, :, 0:HDIM])
                pv = pt.rearrange("p (h d) -> p h d", h=HEADS)
                if SC_V == 1.0:
                    nc.vector.tensor_copy(out=vv, in_=pv)
                else:
                    nc.vector.tensor_scalar_mul(vv, pv, SC_V)

        def emit_attn_j(tl, j, pos):
            # scores for key chunks 2j, 2j+1 (heads 2tl, 2tl+1 packed at
            # partition bases 0/64 -> concurrent PE row groups)
            sss = []
            for hh in range(2):
                s = pp.tile([P, 2, RPC], F32, name=f"sc{tl}_{hh}_{j}",
                            tag=f"sc{hh}", bufs=1)
                sss.append(s)
            for half in range(2):
                i = 2 * j + half
                for hh in range(2):
                    bpart = HDIM * hh
                    nc.tensor.matmul(
                        out=sss[hh][:, half, :],
                        lhsT=kTs[tl][bpart:bpart + HDIM, i * P:(i + 1) * P],
                        rhs=qTs[tl][bpart:bpart + HDIM, :],
                        start=True, stop=True)
            for hh in range(2):
                es = sb.tile([P, 2, RPC], ES_DT, name=f"es{tl}_{hh}_{j}",
                             tag=f"es{hh}", bufs=2)
                nc.scalar.activation(es.rearrange("p a b -> p (a b)"),
                                     sss[hh].rearrange("p a b -> p (a b)"),
                                     AF.Exp, scale=0.125)
                h = 2 * tl + hh
                if ATTN_FP8:
                    nc.tensor.matmul(
                        out=pos[hh],
                        lhsT=vpd[j][:, :, h * VSLOT:h * VSLOT + HDIM + 1],
                        rhs=es,
                        start=(j == 0), stop=(j == NPAIR - 1),
                        perf_mode=DR)
                else:
                    for half in range(2):
                        nc.tensor.matmul(
                            out=pos[hh],
                            lhsT=vpd[j][:, half,
                                        h * VSLOT:h * VSLOT + HDIM + 1],
                            rhs=es[:, half, :],
                            start=(j == 0 and half == 0),
                            stop=(j == NPAIR - 1 and half == 1))

        def emit_division(tl, pos):
            # 1/D via 2-step Newton from constant seed (D ~ 8*4096 +- few %).
            # D rows live at partitions 0 and 32 (engines need quarter-
            # aligned start partitions).
            d2 = sb.tile([33, RPC], F32, name=f"d2_{tl}", tag="d2", bufs=2)
            nc.gpsimd.memset(d2, 1.0 / REC_SEED)
            for hh in range(2):
                nc.vector.tensor_copy(out=d2[32 * hh:32 * hh + 1, :],
                                      in_=pos[hh][HDIM:HDIM + 1, :])
            y1 = sb.tile([33, RPC], F32, name=f"y1_{tl}", tag="y1", bufs=2)
            nc.vector.tensor_scalar(y1, d2, -REC_SEED * REC_SEED,
                                    2.0 * REC_SEED, op0=ALU.mult, op1=ALU.add)
            tt = sb.tile([33, RPC], F32, name=f"tt_{tl}", tag="tt", bufs=2)
            nc.vector.scalar_tensor_tensor(tt, in0=y1, scalar=1.0, in1=d2,
                                           op0=ALU.mult, op1=ALU.mult)
            uu = sb.tile([33, RPC], F32, name=f"uu_{tl}", tag="uu", bufs=2)
            nc.vector.tensor_scalar(uu, tt, -1.0, 2.0, op0=ALU.mult,
                                    op1=ALU.add)
            for hh in range(2):
                # separate base-partition-0 tiles (matmul rhs must align
                # with lhsT's base partition)
                y2 = sb.tile([1, RPC], BF, name=f"y2_{tl}_{hh}", tag="y2",
                             bufs=4)
                nc.vector.tensor_tensor(out=y2,
                                        in0=uu[32 * hh:32 * hh + 1, :],
                                        in1=y1[32 * hh:32 * hh + 1, :],
                                        op=ALU.mult)
                onum = sb.tile([HDIM, RPC], F32, name=f"onum{tl}_{hh}",
                               tag="onum", bufs=2)
                nc.vector.tensor_copy(out=onum, in_=pos[hh][0:HDIM, :])
                pb = pp.tile([HDIM, RPC], F32, name=f"pb{tl}_{hh}", tag="mm",
                             bufs=2)
                nc.tensor.matmul(out=pb, lhsT=ones_bf[0:1, 0:HDIM], rhs=y2,
                                 start=True, stop=True)
                nc.vector.tensor_tensor(
                    out=oTs[tl][HDIM * hh:HDIM * (hh + 1), :],
                    in0=pb, in1=onum, op=ALU.mult)

        def emit_conv1_m(m):
            # psum = sum_src cnt * H1'[src] (+ rank-1 bias 4*b1 (x) 1/dinv)
            pt = pp.tile([P, RPC], F32, name=f"c1p{m}", tag="mm", bufs=2)
            for jj in range(NPAIR):
                nc.tensor.matmul(out=pt,
                                 lhsT=H1f8[jj][:, :, m * P:(m + 1) * P],
                                 rhs=aTs[:, 2 * jj:2 * jj + 2, :],
                                 start=(jj == 0), stop=False,
                                 perf_mode=DR)
            nc.tensor.matmul(out=pt, lhsT=bfr[0:1, m * P:(m + 1) * P],
                             rhs=bfr[0:1, 2 * HID:3 * HID],
                             start=False, stop=True)
            t = sb.tile([P, RPC], BF, name=f"h1T{m}", tag=f"h1T{m}")
            nc.vector.tensor_scalar_max(t, pt, 0.0)
            h1Ts.append(t)

        def emit_conv2_m(m, H2f8):
            # even node-pairs first (AllGather chunk 0), then odd (chunk 1)
            pt = pp.tile([P, RPC], F32, name=f"c2p{m}", tag="mm", bufs=2)
            order = list(range(0, NPAIR, 2)) + list(range(1, NPAIR, 2))
            for idx, jj in enumerate(order):
                nc.tensor.matmul(
                    out=pt,
                    lhsT=H2f8[jj][:, :, m * P:(m + 1) * P],
                    rhs=aTs[:, 2 * jj:2 * jj + 2, :],
                    start=(idx == 0), stop=False,
                    perf_mode=DR)
            nc.tensor.matmul(out=pt, lhsT=bfr[0:1, HID + m * P:HID + (m + 1) * P],
                             rhs=bfr[0:1, 2 * HID:3 * HID],
                             start=False, stop=True)
            t = sb.tile([P, RPC], BF, name=f"h2T{m}", tag=f"h2T{m}")
            nc.vector.tensor_scalar_max(t, pt, 0.0)
            h2Ts.append(t)

        # H2pre AllGather, split by NODE-ROW halves: chunk f holds every
        # core's rows f*256..f*256+256, so chunk f serves node pairs with
        # jj % 2 == f. Issued right after its two H2p m-tiles so the
        # collective flies during the tl0 tail + tl1 pass.
        FCH = 2
        agi_h2 = [dr.tile([RPC // 2, HID], F8, name=f"agi_h2_{f}",
                          tag=f"agi_h2_{f}") for f in range(FCH)]
        ago_h2 = [dr.tile([N_NODES // 2, HID], F8, name=f"ago_h2_{f}",
                          tag=f"ago_h2_{f}", addr_space="Shared")
                  for f in range(FCH)]
        H2f8 = [None] * NPAIR

        def emit_h2p_m(m):
            pt = pp.tile([P, HID], F32, name=f"h2p{m}", tag="mm", bufs=2)
            for k in range(KCH_HID):
                nc.tensor.matmul(out=pt, lhsT=h1Ts[k][:, m * P:(m + 1) * P],
                                 rhs=w2s[k], start=(k == 0),
                                 stop=(k == KCH_HID - 1))
            hc = sb.tile([P, HID], F8, name=f"hc{m}", tag="hc", bufs=2)
            nc.vector.tensor_scalar_mul(hc, pt, d16c[m])
            nc.sync.dma_start(out=agi_h2[m // 2][(m % 2) * P:(m % 2 + 1) * P, :],
                              in_=hc)

        def emit_ag_h2(f):
            nc.gpsimd.collective_compute(
                "AllGather", ALU.bypass, replica_groups=[list(range(NC))],
                ins=[agi_h2[f].opt()], outs=[ago_h2[f].opt()])
            for jj in range(f, NPAIR, 2):
                t = sb.tile([P, 2, HID], F8, name=f"H2f8_{jj}",
                            tag=f"H2f8_{jj}")
                nc.sync.dma_start(
                    out=t,
                    in_=ago_h2[f][(jj // 2) * 2 * P:(jj // 2 + 1) * 2 * P, :]
                        .rearrange("(s p) f -> p s f", p=P))
                H2f8[jj] = t

        # ---------------- tl=0 pass: H1 + kT/v + attention + conv1 + H2p ---
        pos0 = [pp.tile([HDIM + 1, RPC], F32, name=f"ob0_{hh}", tag="ob",
                        bufs=2) for hh in range(2)]
        for n in range(8):
            emit_kv_chunk(n)
            if n < 2:
                for i in range(16 * n, 16 * n + 16):
                    emit_h1_tile(i)
            for j in (2 * n, 2 * n + 1):
                emit_attn_j(0, j, pos0)
            if 2 <= n < 6:
                emit_conv1_m(n - 2)
            if n == 5:
                # issue the gathers as early as the dependency chain allows;
                # they fly under the attention tail + tl1 pass
                for m in range(MT_Q):
                    emit_h2p_m(m)
                emit_ag_h2(0)
                emit_ag_h2(1)
        emit_division(0, pos0)

        # Pc = opw^T @ pw (needed only at the final projection)
        Pcs = []
        for m in range(2):
            pt = pp.tile([P, CLS], F32, name=f"pc_ps{m}", tag="mm", bufs=2)
            for k in range(KCH_IN):
                nc.tensor.matmul(out=pt, lhsT=opws[k][:, m * P:(m + 1) * P],
                                 rhs=pws[k], start=(k == 0), stop=(k == KCH_IN - 1))
            t = sb.tile([P, CLS], BF, name=f"Pcs{m}", tag=f"Pcs{m}")
            nc.vector.tensor_copy(out=t, in_=pt)
            Pcs.append(t)

        # ---------------- tl=1 pass hosting conv2 --------------------------
        pos1 = [pp.tile([HDIM + 1, RPC], F32, name=f"ob1_{hh}", tag="ob",
                        bufs=2) for hh in range(2)]
        for j in range(NPAIR):
            emit_attn_j(1, j, pos1)
            if j in (8, 10, 12, 14):
                emit_conv2_m((j - 8) // 2, H2f8)

        # x_gnn evacs can run before the tl1 division completes
        xgss = []
        for m in range(MT_Q):
            pg = pp.tile([P, CLS], F32, name=f"xg_ps{m}", tag="mm", bufs=2)
            for k in range(KCH_HID):
                nc.tensor.matmul(out=pg, lhsT=h2Ts[k][:, m * P:(m + 1) * P],
                                 rhs=lws[k], start=(k == 0),
                                 stop=(k == KCH_HID - 1))
            xgs = sb.tile([P, CLS], F32, name=f"xgs{m}", tag=f"xgs{m}")
            nc.vector.tensor_scalar_mul(xgs, pg, dfc[m])
            xgss.append(xgs)

        emit_division(1, pos1)

        # ---------------- final: x_gnn + x_proj, relu, store ---------------
        for m in range(MT_Q):
            pj = pp.tile([P, CLS], F32, name=f"xp_ps{m}", tag="mm", bufs=2)
            for k in range(2):
                nc.tensor.matmul(out=pj, lhsT=oTs[k][:, m * P:(m + 1) * P],
                                 rhs=Pcs[k], start=(k == 0), stop=False)
            nc.tensor.matmul(out=pj, lhsT=ones_bf[0:1, 0:P], rhs=brows,
                             start=False, stop=True)
            tadd = sb.tile([P, CLS], F32, name=f"tadd{m}", tag="tadd", bufs=2)
            nc.vector.scalar_tensor_tensor(tadd, in0=pj, scalar=0.0,
                                           in1=xgss[m], op0=ALU.add,
                                           op1=ALU.add)
            osb = sb.tile([P, CLS], F32, name=f"osb{m}", tag="osb", bufs=2)
            nc.vector.tensor_scalar_max(osb, tadd, 0.0)
            nc.sync.dma_start(out=out[m * P:(m + 1) * P, :], in_=osb)


_CACHE = {}


def _get_compiled():
    if "nc" not in _CACHE:
        nc = bacc.Bacc("TRN2", target_bir_lowering=False, debug=False,
                       num_devices=NC)
        with tile.TileContext(nc) as tc:
            _emit(tc)
        nc.compile()
        _CACHE["nc"] = nc
    return _CACHE["nc"]


def _prepare_in_maps(inputs):
    bf16 = ml_dtypes.bfloat16
    fp8 = ml_dtypes.float8_e4m3
    x = np.asarray(inputs["x"], dtype=np.float32)
    ei = np.asarray(inputs["edge_index"]).astype(np.int64)

    loop = np.arange(N_NODES, dtype=np.int64)
    src = np.concatenate([ei[0], loop])
    dst = np.concatenate([ei[1], loop])
    deg = np.bincount(dst, minlength=N_NODES).astype(np.float64)
    dinv = np.where(deg > 0, 1.0 / np.sqrt(deg), 0.0).astype(np.float32)
    # integer edge-count matrix: A = dinv[dst] * cnt * dinv[src], cnt exact
    cnt = np.bincount(dst * N_NODES + src,
                      minlength=N_NODES * N_NODES).astype(np.float32)
    cnt = cnt.reshape(N_NODES, N_NODES)

    xT = np.ascontiguousarray(x.T).astype(bf16)
    w1 = np.asarray(inputs["gcn1_w"], np.float32).astype(bf16)
    # h1T tiles carry (4/dinv)*h1, h2T carry (16/dinv)*h2
    w2 = (np.asarray(inputs["gcn2_w"], np.float32) / SC_H1).astype(bf16)
    lwv = (np.asarray(inputs["lin_w"], np.float32) / SC_H2).astype(bf16)
    ipT = np.ascontiguousarray(
        np.asarray(inputs["in_proj_w"], np.float32).T).astype(bf16)
    opwv = np.asarray(inputs["out_proj_w"], np.float32).astype(bf16)
    pwv = np.asarray(inputs["proj_w"], np.float32).astype(bf16)

    b1 = np.asarray(inputs["gcn1_b"], np.float32) * SC_H1
    b2 = np.asarray(inputs["gcn2_b"], np.float32) * SC_H2
    ipb = np.asarray(inputs["in_proj_b"], np.float32)
    # bv folded through out_proj/proj: softmax weights sum to 1, so the v
    # bias reaches the output as a constant row
    bv = ipb[2 * IN_DIM:3 * IN_DIM]
    opwf = np.asarray(inputs["out_proj_w"], np.float32)
    pwf = np.asarray(inputs["proj_w"], np.float32)
    bprow = (np.asarray(inputs["lin_b"], np.float32)
             + (np.asarray(inputs["out_proj_b"], np.float32) + bv @ opwf.T)
             @ pwf
             + np.asarray(inputs["proj_b"], np.float32))
    browv = np.ascontiguousarray(bprow[None, :]).astype(bf16)

    in_maps = []
    for c in range(NC):
        sl = slice(c * RPC, (c + 1) * RPC)
        dc = dinv[sl]
        bpk = np.zeros((P, 14), np.float32)
        bpk[:, 0:4] = (SC_H1 * dc).reshape(4, P).T
        bpk[:, 4:8] = (SC_H2 * dc * dc).reshape(4, P).T
        bpk[:, 8:12] = dc.reshape(4, P).T
        bpk[:, 12:14] = ipb[0:IN_DIM].reshape(2, P).T
        bpdk = np.ascontiguousarray((SC_H1 * dinv).reshape(NT, P).T)
        bfrk = np.zeros((1, 3 * HID), np.float32)
        bfrk[0, 0:HID] = b1
        bfrk[0, HID:2 * HID] = b2
        bfrk[0, 2 * HID:3 * HID] = 1.0 / dc
        aTc = np.ascontiguousarray(cnt[sl, :].T)  # [src, dst_local]
        aT8 = np.clip(aTc, 0.0, 240.0).astype(fp8)
        aTd = np.ascontiguousarray(
            aT8.reshape(NT, P, RPC).transpose(1, 0, 2))
        in_maps.append({
            "xT": xT,
            "xq": np.ascontiguousarray(xT[:, sl]),
            "aTd": aTd,
            "w1": w1, "w2": w2, "lw": lwv, "ipT": ipT,
            "opw": opwv, "pw": pwv,
            "bp": bpk, "bpd": bpdk, "bfrd": bfrk.astype(bf16),
            "brow": browv,
        })
    return in_maps


def _run(inputs, trace=False):
    nc = _get_compiled()
    in_maps = _prepare_in_maps(inputs)
    res = run_bass_kernel_spmd(nc, in_maps, core_ids=list(range(NC)),
                               trace=trace)
    out = np.concatenate([res.results[c]["out"] for c in range(NC)], axis=0)
    return np.ascontiguousarray(out.astype(np.float32)), res


def kernel(**inputs):
    out, _ = _run(inputs, trace=False)
    return out


# revision 48
# speedup vs baseline: 1.1278x; 1.1278x over previous
"""GraphTransformer (2x GCNConv + global MHA) on 8 TRN2 NeuronCores.

Strategy (v2)
-------------
Nodes (N=4096) are sharded 512/core. The GCN scatter-add is a dense SpMM
against the normalized adjacency A (built on host from edge_index; pure index
preprocessing). All heavy matmuls that tolerate it run in fp8e4 with
DoubleRow double-pumping (2 MACs/cell/cycle):

  - conv1/conv2: lhsT = H (node-pair fp8 tiles [128,2,F]), rhs = A^T pair
    tiles [128,2,512]. A is pre-scaled x8, H1 x4, H2pre x16 on device; the
    scales are folded into the (host-scaled) weights / bias adds.
  - attn@V: lhsT = V' pairs [128,2,80h+65], rhs = exp(scores) fp8.

H1 = X@W1 is computed sharded (512 rows/core) and AllGathered in fp8 (2 MB),
as is H2pre. K-bias is dropped (softmax is invariant to per-query shifts).
conv relu+bias evacuations run on DVE (tensor_scalar add+max) with weight
scaling folded on host, keeping ACT free for the 64 softmax EXPs (the ~73 us
scalar-engine floor). Emission interleaves conv matmuls into the attention
j-loop so the PE fills the exp-stall gaps; softmax division uses a 2-step
Newton reciprocal on DVE from a constant seed.
"""

import os
import sys

import numpy as np
import ml_dtypes

try:
    import concourse  # noqa: F401
except ImportError:  # pragma: no cover
    sys.path.insert(0, "/opt/trn_rl_repo")

from concourse import bacc, bass, mybir, tile
from concourse.bass_utils import run_bass_kernel_spmd

P = 128
N_NODES = 4096
E_EDGES = 131072
IN_DIM = 256
HID = 512
CLS = 256
HEADS = 4
HDIM = 64
NC = 8
RPC = N_NODES // NC  # 512 rows per core

BF = mybir.dt.bfloat16
F32 = mybir.dt.float32
F32R = mybir.dt.float32r
F8 = mybir.dt.float8e4
AF = mybir.ActivationFunctionType
ALU = mybir.AluOpType
DR = mybir.MatmulPerfMode.DoubleRow

KCH_IN = IN_DIM // P    # 2
KCH_HID = HID // P      # 4
NT = N_NODES // P       # 32 node tiles
NPAIR = NT // 2         # 16 node-tile pairs
MT_Q = RPC // P         # 4 query tiles per core

# The adjacency factors exactly: A = dinv[dst] * cnt * dinv[src] with integer
# cnt (exact in fp8). dinv scalings ride per-partition evac scales + rank-1
# bias matmuls, so the conv matmuls see NO A-quantization error.
SC_H1 = 4.0      # H1' = 4*dinv*H1 in fp8
SC_H2 = 16.0     # hc = 16*dinv^2*H2pre in fp8
ATTN_FP8 = False  # es/v' in fp8 + DoubleRow attn@V (faster, more error)
SC_V = 8.0 if ATTN_FP8 else 1.0
ES_DT = mybir.dt.float8e4 if ATTN_FP8 else mybir.dt.bfloat16
# Newton seed for 1/D, D = SC_V * sum_k exp(s) ~ SC_V*4096
REC_SEED = 1.0 / (4096.0 * SC_V)

VSLOT = 80  # per-head slot in v' tiles (64 v dims + 1 ones + pad, 16B aligned)


def _emit(tc):
    nc = tc.nc

    # ---------------- I/O ----------------
    xT = nc.dram_tensor("xT", [IN_DIM, N_NODES], BF, kind="ExternalInput")
    xq = nc.dram_tensor("xq", [IN_DIM, RPC], BF, kind="ExternalInput")
    aTd = nc.dram_tensor("aTd", [P, NT, RPC], F8, kind="ExternalInput")
    w1 = nc.dram_tensor("w1", [IN_DIM, HID], BF, kind="ExternalInput")
    w2 = nc.dram_tensor("w2", [HID, HID], BF, kind="ExternalInput")
    lw = nc.dram_tensor("lw", [HID, CLS], BF, kind="ExternalInput")
    ipT = nc.dram_tensor("ipT", [IN_DIM, 3 * IN_DIM], BF, kind="ExternalInput")
    opw = nc.dram_tensor("opw", [IN_DIM, IN_DIM], BF, kind="ExternalInput")
    pw = nc.dram_tensor("pw", [IN_DIM, CLS], BF, kind="ExternalInput")
    bp = nc.dram_tensor("bp", [P, 14], F32, kind="ExternalInput")
    bpd = nc.dram_tensor("bpd", [P, NT], F32, kind="ExternalInput")
    bfrd = nc.dram_tensor("bfrd", [1, 3 * HID], BF, kind="ExternalInput")
    brow = nc.dram_tensor("brow", [1, CLS], BF, kind="ExternalInput")
    out = nc.dram_tensor("out", [RPC, CLS], F32, kind="ExternalOutput")

    from contextlib import ExitStack

    with ExitStack() as ctx:
        sb = ctx.enter_context(tc.tile_pool(name="sb", bufs=1))
        pp = ctx.enter_context(tc.tile_pool(name="pp", bufs=1, space="PSUM"))
        dr = ctx.enter_context(tc.tile_pool(name="dr", bufs=1, space="DRAM"))

        # ---------------- input DMA --------------------------------------
        # H1 needs w1 + both xT halves first; kT/v need ipT next. Split the
        # big early tensors across the two DMA queues; late-consumed tensors
        # (aTs, w2, opw, pw, lw) go after.
        xTs, ipTs = [], []
        for k in range(KCH_IN):
            t = sb.tile([P, N_NODES], BF, name=f"xTs{k}", tag=f"xTs{k}")
            xTs.append(t)
            t2 = sb.tile([P, 3 * IN_DIM], BF, name=f"ipTs{k}", tag=f"ipTs{k}")
            ipTs.append(t2)
        w1s = []
        for k in range(KCH_IN):
            t = sb.tile([P, HID], BF, name=f"w1s{k}", tag=f"w1s{k}")
            nc.gpsimd.dma_start(out=t, in_=w1[k * P:(k + 1) * P, :])
            w1s.append(t)
        nc.sync.dma_start(out=xTs[0], in_=xT[0:P, :])
        nc.gpsimd.dma_start(out=xTs[1], in_=xT[P:2 * P, :])
        nc.sync.dma_start(out=ipTs[0], in_=ipT[0:P, :])
        nc.gpsimd.dma_start(out=ipTs[1], in_=ipT[P:2 * P, :])
        bps = sb.tile([P, 14], F32, name="bps", tag="bps")
        nc.sync.dma_start(out=bps, in_=bp[:, :])
        bpds = sb.tile([P, NT], F32, name="bpds", tag="bpds")
        nc.sync.dma_start(out=bpds, in_=bpd[:, :])
        bfr = sb.tile([1, 3 * HID], BF, name="bfr", tag="bfr")
        nc.sync.dma_start(out=bfr, in_=bfrd[:, :])
        xqs = []
        for k in range(KCH_IN):
            t = sb.tile([P, RPC], BF, name=f"xqs{k}", tag=f"xqs{k}")
            nc.sync.dma_start(out=t, in_=xq[k * P:(k + 1) * P, :])
            xqs.append(t)
        w2s = []
        for k in range(KCH_HID):
            t = sb.tile([P, HID], BF, name=f"w2s{k}", tag=f"w2s{k}")
            nc.gpsimd.dma_start(out=t, in_=w2[k * P:(k + 1) * P, :])
            w2s.append(t)
        opws, pws = [], []
        for k in range(KCH_IN):
            t = sb.tile([P, IN_DIM], BF, name=f"opws{k}", tag=f"opws{k}")
            nc.gpsimd.dma_start(out=t, in_=opw[k * P:(k + 1) * P, :])
            opws.append(t)
            t2 = sb.tile([P, CLS], BF, name=f"pws{k}", tag=f"pws{k}")
            nc.gpsimd.dma_start(out=t2, in_=pw[k * P:(k + 1) * P, :])
            pws.append(t2)
        brows = sb.tile([1, CLS], BF, name="brows", tag="brows")
        nc.gpsimd.dma_start(out=brows, in_=brow[:, :])
        lws = []
        for k in range(KCH_HID):
            t = sb.tile([P, CLS], BF, name=f"lws{k}", tag=f"lws{k}")
            nc.gpsimd.dma_start(out=t, in_=lw[k * P:(k + 1) * P, :])
            lws.append(t)

        ones_bf = sb.tile([1, P], BF, name="ones_bf", tag="ones_bf")
        nc.vector.memset(ones_bf, 1.0)


        # per-partition scale/bias slices
        d4c = [bps[:, m:m + 1] for m in range(0, 4)]          # 4*dinv (m-tile)
        d16c = [bps[:, 4 + m:5 + m] for m in range(0, 4)]     # 16*dinv^2
        dfc = [bps[:, 8 + m:9 + m] for m in range(0, 4)]      # dinv
        bqc = [bps[:, 12 + m:13 + m] for m in range(0, 2)]    # bq

        # big late-consumed input: adjacency counts (conv1 needs it ~30us in)
        aTs = sb.tile([P, NT, RPC], F8, name="aTs", tag="aTs")
        nc.sync.dma_start(out=aTs, in_=aTd[:, :, :])
        # H1' = 4*dinv*(X @ W1) computed REPLICATED (no collective: the first
        # collective would absorb cross-core launch skew as dead PE time).
        # Node-pair fp8 tiles for the conv1 DoubleRow lhsT, produced in the
        # tl0 loop below (all 32 node tiles done by block n=3).
        H1f8 = [sb.tile([P, 2, HID], F8, name=f"H1f8_{j}", tag=f"H1f8_{j}")
                for j in range(NPAIR)]

        def emit_h1_tile(i):
            pt = pp.tile([P, HID], F32, name=f"h1p{i}", tag="mm", bufs=2)
            for k in range(KCH_IN):
                nc.tensor.matmul(out=pt, lhsT=xTs[k][:, i * P:(i + 1) * P],
                                 rhs=w1s[k], start=(k == 0),
                                 stop=(k == KCH_IN - 1))
            # alternate evacuation between DVE and ACT to balance engines
            if i % 2 == 0:
                nc.vector.tensor_scalar_mul(H1f8[i // 2][:, i % 2, :], pt,
                                            bpds[:, i:i + 1])
            else:
                nc.scalar.activation(H1f8[i // 2][:, i % 2, :], pt, AF.Copy,
                                     scale=bpds[:, i:i + 1])

        # ---------------- Phase B: qT (+bq) ---------------------------------
        qTs = []
        for m in range(2):
            pt = pp.tile([P, RPC], F32, name=f"q_ps{m}", tag="mm", bufs=2)
            for k in range(KCH_IN):
                nc.tensor.matmul(out=pt, lhsT=ipTs[k][:, m * P:(m + 1) * P],
                                 rhs=xqs[k], start=(k == 0), stop=(k == KCH_IN - 1))
            t = sb.tile([P, RPC], BF, name=f"qTs{m}", tag=f"qTs{m}")
            nc.vector.tensor_scalar_add(t, pt, bqc[m])
            qTs.append(t)

        # persistent attention state
        kTs = [sb.tile([P, N_NODES], BF, name=f"kTs{m}", tag=f"kTs{m}")
               for m in range(2)]
        vpd = [sb.tile([P, 2, HEADS * VSLOT], ES_DT, name=f"vpd{j}",
                       tag=f"vpd{j}")
               for j in range(NPAIR)]
        # ones columns for the softmax denominator (only the 8 columns the
        # attn@V lhsT slice actually reads; the pad columns stay untouched)
        for j in range(NPAIR):
            nc.gpsimd.memset(
                vpd[j].rearrange("p s (h c) -> p s h c", c=VSLOT)
                [:, :, :, HDIM:HDIM + 1], SC_V)
        oTs = [sb.tile([P, RPC], BF, name=f"oTs{m}", tag=f"oTs{m}")
               for m in range(2)]
        h1Ts, h2Ts = [], []

        def emit_kv_chunk(n):
            # kT columns n*512 .. +512 (both head-pair tiles), no k-bias
            # (softmax is invariant to per-query score shifts).
            for tl in range(2):
                pt = pp.tile([P, 512], F32, name=f"k_ps{tl}_{n}", tag="mm",
                             bufs=2)
                for k in range(KCH_IN):
                    nc.tensor.matmul(
                        out=pt,
                        lhsT=ipTs[k][:, IN_DIM + tl * P:IN_DIM + (tl + 1) * P],
                        rhs=xTs[k][:, n * 512:(n + 1) * 512],
                        start=(k == 0), stop=(k == KCH_IN - 1))
                nc.vector.tensor_copy(out=kTs[tl][:, n * 512:(n + 1) * 512],
                                      in_=pt)
            # v' for node tiles 4n..4n+3 (pairs 2n, 2n+1); the v bias is
            # folded into brow on the host (softmax weights sum to 1, so
            # bv rides through out_proj @ proj as a constant row).
            for i in range(4 * n, 4 * n + 4):
                pt = pp.tile([P, IN_DIM], F32, name=f"v_ps{i}", tag="mm",
                             bufs=2)
                for k in range(KCH_IN):
                    nc.tensor.matmul(out=pt, lhsT=xTs[k][:, i * P:(i + 1) * P],
                                     rhs=ipTs[k][:, 2 * IN_DIM:3 * IN_DIM],
                                     start=(k == 0), stop=(k == KCH_IN - 1))
                vv = (vpd[i // 2][:, i % 2, :]
                      .rearrange("p (h d) -> p h d", h=HEADS)[:, :, 0:HDIM])
                pv = pt.rearrange("p (h d) -> p h d", h=HEADS)
                if SC_V == 1.0:
                    nc.vector.tensor_copy(out=vv, in_=pv)
                else:
                    nc.vector.tensor_scalar_mul(vv, pv, SC_V)

        def emit_attn_j(tl, j, pos):
            # scores for key chunks 2j, 2j+1 (heads 2tl, 2tl+1 packed at
            # partition bases 0/64 -> concurrent PE row groups)
            sss = []
            for hh in range(2):
                s = pp.tile([P, 2, RPC], F32, name=f"sc{tl}_{hh}_{j}",
                            tag=f"sc{hh}", bufs=1)
                sss.append(s)
            for half in range(2):
                i = 2 * j + half
                for hh in range(2):
                    bpart = HDIM * hh
                    nc.tensor.matmul(
                        out=sss[hh][:, half, :],
                        lhsT=kTs[tl][bpart:bpart + HDIM, i * P:(i + 1) * P],
                        rhs=qTs[tl][bpart:bpart + HDIM, :],
                        start=True, stop=True)
            for hh in range(2):
                es = sb.tile([P, 2, RPC], ES_DT, name=f"es{tl}_{hh}_{j}",
                             tag=f"es{hh}", bufs=2)
                nc.scalar.activation(es.rearrange("p a b -> p (a b)"),
                                     sss[hh].rearrange("p a b -> p (a b)"),
                                     AF.Exp, scale=0.125)
                h = 2 * tl + hh
                if ATTN_FP8:
                    nc.tensor.matmul(
                        out=pos[hh],
                        lhsT=vpd[j][:, :, h * VSLOT:h * VSLOT + HDIM + 1],
                        rhs=es,
                        start=(j == 0), stop=(j == NPAIR - 1),
                        perf_mode=DR)
                else:
                    for half in range(2):
                        nc.tensor.matmul(
                            out=pos[hh],
                            lhsT=vpd[j][:, half,
                                        h * VSLOT:h * VSLOT + HDIM + 1],
                            rhs=es[:, half, :],
                            start=(j == 0 and half == 0),
                            stop=(j == NPAIR - 1 and half == 1))

        def emit_division(tl, pos):
            # 1/D via 2-step Newton from constant seed (D ~ 8*4096 +- few %).
            # D rows live at partitions 0 and 32 (engines need quarter-
            # aligned start partitions).
            d2 = sb.tile([33, RPC], F32, name=f"d2_{tl}", tag="d2", bufs=2)
            nc.gpsimd.memset(d2, 1.0 / REC_SEED)
            for hh in range(2):
                nc.vector.tensor_copy(out=d2[32 * hh:32 * hh + 1, :],
                                      in_=pos[hh][HDIM:HDIM + 1, :])
            y1 = sb.tile([33, RPC], F32, name=f"y1_{tl}", tag="y1", bufs=2)
            nc.vector.tensor_scalar(y1, d2, -REC_SEED * REC_SEED,
                                    2.0 * REC_SEED, op0=ALU.mult, op1=ALU.add)
            tt = sb.tile([33, RPC], F32, name=f"tt_{tl}", tag="tt", bufs=2)
            nc.vector.scalar_tensor_tensor(tt, in0=y1, scalar=1.0, in1=d2,
                                           op0=ALU.mult, op1=ALU.mult)
            uu = sb.tile([33, RPC], F32, name=f"uu_{tl}", tag="uu", bufs=2)
            nc.vector.tensor_scalar(uu, tt, -1.0, 2.0, op0=ALU.mult,
                                    op1=ALU.add)
            for hh in range(2):
                # separate base-partition-0 tiles (matmul rhs must align
                # with lhsT's base partition)
                y2 = sb.tile([1, RPC], BF, name=f"y2_{tl}_{hh}", tag="y2",
                             bufs=4)
                nc.vector.tensor_tensor(out=y2,
                                        in0=uu[32 * hh:32 * hh + 1, :],
                                        in1=y1[32 * hh:32 * hh + 1, :],
                                        op=ALU.mult)
                onum = sb.tile([HDIM, RPC], F32, name=f"onum{tl}_{hh}",
                               tag="onum", bufs=2)
                nc.vector.tensor_copy(out=onum, in_=pos[hh][0:HDIM, :])
                pb = pp.tile([HDIM, RPC], F32, name=f"pb{tl}_{hh}", tag="mm",
                             bufs=2)
                nc.tensor.matmul(out=pb, lhsT=ones_bf[0:1, 0:HDIM], rhs=y2,
                                 start=True, stop=True)
                nc.vector.tensor_tensor(
                    out=oTs[tl][HDIM * hh:HDIM * (hh + 1), :],
                    in0=pb, in1=onum, op=ALU.mult)

        def emit_conv1_m(m):
            # psum = sum_src cnt * H1'[src] (+ rank-1 bias 4*b1 (x) 1/dinv)
            pt = pp.tile([P, RPC], F32, name=f"c1p{m}", tag="mm", bufs=2)
            for jj in range(NPAIR):
                nc.tensor.matmul(out=pt,
                                 lhsT=H1f8[jj][:, :, m * P:(m + 1) * P],
                                 rhs=aTs[:, 2 * jj:2 * jj + 2, :],
                                 start=(jj == 0), stop=False,
                                 perf_mode=DR)
            nc.tensor.matmul(out=pt, lhsT=bfr[0:1, m * P:(m + 1) * P],
                             rhs=bfr[0:1, 2 * HID:3 * HID],
                             start=False, stop=True)
            t = sb.tile([P, RPC], BF, name=f"h1T{m}", tag=f"h1T{m}")
            nc.vector.tensor_scalar_max(t, pt, 0.0)
            h1Ts.append(t)

        def emit_conv2_m(m, H2f8):
            # even node-pairs first (AllGather chunk 0), then odd (chunk 1)
            pt = pp.tile([P, RPC], F32, name=f"c2p{m}", tag="mm", bufs=2)
            order = list(range(0, NPAIR, 2)) + list(range(1, NPAIR, 2))
            for idx, jj in enumerate(order):
                nc.tensor.matmul(
                    out=pt,
                    lhsT=H2f8[jj][:, :, m * P:(m + 1) * P],
                    rhs=aTs[:, 2 * jj:2 * jj + 2, :],
                    start=(idx == 0), stop=False,
                    perf_mode=DR)
            nc.tensor.matmul(out=pt, lhsT=bfr[0:1, HID + m * P:HID + (m + 1) * P],
                             rhs=bfr[0:1, 2 * HID:3 * HID],
                             start=False, stop=True)
            t = sb.tile([P, RPC], BF, name=f"h2T{m}", tag=f"h2T{m}")
            nc.vector.tensor_scalar_max(t, pt, 0.0)
            h2Ts.append(t)

        # H2pre AllGather (single collective; conv2 sits at the very end of
        # the PE stream so a slow collective can never head-of-line block
        # the attention matmuls behind it)
        agi_h2 = dr.tile([RPC, HID], F8, name="agi_h2", tag="agi_h2")
        ago_h2 = dr.tile([N_NODES, HID], F8, name="ago_h2", tag="ago_h2",
                         addr_space="Shared")
        H2f8 = [None] * NPAIR

        def emit_h2p_m(m):
            pt = pp.tile([P, HID], F32, name=f"h2p{m}", tag="mm", bufs=2)
            for k in range(KCH_HID):
                nc.tensor.matmul(out=pt, lhsT=h1Ts[k][:, m * P:(m + 1) * P],
                                 rhs=w2s[k], start=(k == 0),
                                 stop=(k == KCH_HID - 1))
            hc = sb.tile([P, HID], F8, name=f"hc{m}", tag="hc", bufs=2)
            nc.vector.tensor_scalar_mul(hc, pt, d16c[m])
            nc.sync.dma_start(out=agi_h2[m * P:(m + 1) * P, :], in_=hc)

        def emit_ag_h2():
            nc.gpsimd.collective_compute(
                "AllGather", ALU.bypass, replica_groups=[list(range(NC))],
                ins=[agi_h2.opt()], outs=[ago_h2.opt()])
            for jj in range(NPAIR):
                t = sb.tile([P, 2, HID], F8, name=f"H2f8_{jj}",
                            tag=f"H2f8_{jj}")
                nc.sync.dma_start(
                    out=t,
                    in_=ago_h2[jj * 2 * P:(jj + 1) * 2 * P, :]
                        .rearrange("(s p) f -> p s f", p=P))
                H2f8[jj] = t

        # ---------------- GNN critical chain first (high priority) ---------
        # H1 -> conv1 -> H2p -> AllGather; the attention below is emitted
        # later so the scheduler weaves it into this chain's stall gaps.
        for i in range(NT):
            emit_h1_tile(i)
        for m in range(MT_Q):
            emit_conv1_m(m)
        for m in range(MT_Q):
            emit_h2p_m(m)
        emit_ag_h2()

        # ---------------- attention (fills the PE gaps) --------------------
        pos0 = [pp.tile([HDIM + 1, RPC], F32, name=f"ob0_{hh}", tag="ob",
                        bufs=2) for hh in range(2)]
        for n in range(8):
            emit_kv_chunk(n)
            for j in (2 * n, 2 * n + 1):
                emit_attn_j(0, j, pos0)
        emit_division(0, pos0)

        # Pc = opw^T @ pw (needed only at the final projection)
        Pcs = []
        for m in range(2):
            pt = pp.tile([P, CLS], F32, name=f"pc_ps{m}", tag="mm", bufs=2)
            for k in range(KCH_IN):
                nc.tensor.matmul(out=pt, lhsT=opws[k][:, m * P:(m + 1) * P],
                                 rhs=pws[k], start=(k == 0), stop=(k == KCH_IN - 1))
            t = sb.tile([P, CLS], BF, name=f"Pcs{m}", tag=f"Pcs{m}")
            nc.vector.tensor_copy(out=t, in_=pt)
            Pcs.append(t)

        pos1 = [pp.tile([HDIM + 1, RPC], F32, name=f"ob1_{hh}", tag="ob",
                        bufs=2) for hh in range(2)]
        for j in range(NPAIR):
            emit_attn_j(1, j, pos1)

        emit_division(1, pos1)

        # ---------------- conv2 last (never blocks attention) --------------
        for m in range(MT_Q):
            emit_conv2_m(m, H2f8)

        xgss = []
        for m in range(MT_Q):
            pg = pp.tile([P, CLS], F32, name=f"xg_ps{m}", tag="mm", bufs=2)
            for k in range(KCH_HID):
                nc.tensor.matmul(out=pg, lhsT=h2Ts[k][:, m * P:(m + 1) * P],
                                 rhs=lws[k], start=(k == 0),
                                 stop=(k == KCH_HID - 1))
            xgs = sb.tile([P, CLS], F32, name=f"xgs{m}", tag=f"xgs{m}")
            nc.vector.tensor_scalar_mul(xgs, pg, dfc[m])
            xgss.append(xgs)

        # ---------------- final: x_gnn + x_proj, relu, store ---------------
        for m in range(MT_Q):
            pj = pp.tile([P, CLS], F32, name=f"xp_ps{m}", tag="mm", bufs=2)
            for k in range(2):
                nc.tensor.matmul(out=pj, lhsT=oTs[k][:, m * P:(m + 1) * P],
                                 rhs=Pcs[k], start=(k == 0), stop=False)
            nc.tensor.matmul(out=pj, lhsT=ones_bf[0:1, 0:P], rhs=brows,
                             start=False, stop=True)
            tadd = sb.tile([P, CLS], F32, name=f"tadd{m}", tag="tadd", bufs=2)
            nc.vector.scalar_tensor_tensor(tadd, in0=pj, scalar=0.0,
                                           in1=xgss[m], op0=ALU.add,
                                           op1=ALU.add)
            osb = sb.tile([P, CLS], F32, name=f"osb{m}", tag="osb", bufs=2)
            nc.vector.tensor_scalar_max(osb, tadd, 0.0)
            nc.sync.dma_start(out=out[m * P:(m + 1) * P, :], in_=osb)


_CACHE = {}


def _get_compiled():
    if "nc" not in _CACHE:
        nc = bacc.Bacc("TRN2", target_bir_lowering=False, debug=False,
                       num_devices=NC)
        with tile.TileContext(nc) as tc:
            _emit(tc)
        nc.compile()
        _CACHE["nc"] = nc
    return _CACHE["nc"]


def _prepare_in_maps(inputs):
    bf16 = ml_dtypes.bfloat16
    fp8 = ml_dtypes.float8_e4m3
    x = np.asarray(inputs["x"], dtype=np.float32)
    ei = np.asarray(inputs["edge_index"]).astype(np.int64)

    loop = np.arange(N_NODES, dtype=np.int64)
    src = np.concatenate([ei[0], loop])
    dst = np.concatenate([ei[1], loop])
    deg = np.bincount(dst, minlength=N_NODES).astype(np.float64)
    dinv = np.where(deg > 0, 1.0 / np.sqrt(deg), 0.0).astype(np.float32)
    # integer edge-count matrix: A = dinv[dst] * cnt * dinv[src], cnt exact
    cnt = np.bincount(dst * N_NODES + src,
                      minlength=N_NODES * N_NODES).astype(np.float32)
    cnt = cnt.reshape(N_NODES, N_NODES)

    xT = np.ascontiguousarray(x.T).astype(bf16)
    w1 = np.asarray(inputs["gcn1_w"], np.float32).astype(bf16)
    # h1T tiles carry (4/dinv)*h1, h2T carry (16/dinv)*h2
    w2 = (np.asarray(inputs["gcn2_w"], np.float32) / SC_H1).astype(bf16)
    lwv = (np.asarray(inputs["lin_w"], np.float32) / SC_H2).astype(bf16)
    ipT = np.ascontiguousarray(
        np.asarray(inputs["in_proj_w"], np.float32).T).astype(bf16)
    opwv = np.asarray(inputs["out_proj_w"], np.float32).astype(bf16)
    pwv = np.asarray(inputs["proj_w"], np.float32).astype(bf16)

    b1 = np.asarray(inputs["gcn1_b"], np.float32) * SC_H1
    b2 = np.asarray(inputs["gcn2_b"], np.float32) * SC_H2
    ipb = np.asarray(inputs["in_proj_b"], np.float32)
    # bv folded through out_proj/proj: softmax weights sum to 1, so the v
    # bias reaches the output as a constant row
    bv = ipb[2 * IN_DIM:3 * IN_DIM]
    opwf = np.asarray(inputs["out_proj_w"], np.float32)
    pwf = np.asarray(inputs["proj_w"], np.float32)
    bprow = (np.asarray(inputs["lin_b"], np.float32)
             + (np.asarray(inputs["out_proj_b"], np.float32) + bv @ opwf.T)
             @ pwf
             + np.asarray(inputs["proj_b"], np.float32))
    browv = np.ascontiguousarray(bprow[None, :]).astype(bf16)

    in_maps = []
    for c in range(NC):
        sl = slice(c * RPC, (c + 1) * RPC)
        dc = dinv[sl]
        bpk = np.zeros((P, 14), np.float32)
        bpk[:, 0:4] = (SC_H1 * dc).reshape(4, P).T
        bpk[:, 4:8] = (SC_H2 * dc * dc).reshape(4, P).T
        bpk[:, 8:12] = dc.reshape(4, P).T
        bpk[:, 12:14] = ipb[0:IN_DIM].reshape(2, P).T
        bpdk = np.ascontiguousarray((SC_H1 * dinv).reshape(NT, P).T)
        bfrk = np.zeros((1, 3 * HID), np.float32)
        bfrk[0, 0:HID] = b1
        bfrk[0, HID:2 * HID] = b2
        bfrk[0, 2 * HID:3 * HID] = 1.0 / dc
        aTc = np.ascontiguousarray(cnt[sl, :].T)  # [src, dst_local]
        aT8 = np.clip(aTc, 0.0, 240.0).astype(fp8)
        aTd = np.ascontiguousarray(
            aT8.reshape(NT, P, RPC).transpose(1, 0, 2))
        in_maps.append({
            "xT": xT,
            "xq": np.ascontiguousarray(xT[:, sl]),
            "aTd": aTd,
            "w1": w1, "w2": w2, "lw": lwv, "ipT": ipT,
            "opw": opwv, "pw": pwv,
            "bp": bpk, "bpd": bpdk, "bfrd": bfrk.astype(bf16),
            "brow": browv,
        })
    return in_maps


def _run(inputs, trace=False):
    nc = _get_compiled()
    in_maps = _prepare_in_maps(inputs)
    res = run_bass_kernel_spmd(nc, in_maps, core_ids=list(range(NC)),
                               trace=trace)
    out = np.concatenate([res.results[c]["out"] for c in range(NC)], axis=0)
    return np.ascontiguousarray(out.astype(np.float32)), res


def kernel(**inputs):
    out, _ = _run(inputs, trace=False)
    return out


# revision 49
# speedup vs baseline: 1.1385x; 1.0095x over previous
"""GraphTransformer (2x GCNConv + global MHA) on 8 TRN2 NeuronCores.

Strategy (v2)
-------------
Nodes (N=4096) are sharded 512/core. The GCN scatter-add is a dense SpMM
against the normalized adjacency A (built on host from edge_index; pure index
preprocessing). All heavy matmuls that tolerate it run in fp8e4 with
DoubleRow double-pumping (2 MACs/cell/cycle):

  - conv1/conv2: lhsT = H (node-pair fp8 tiles [128,2,F]), rhs = A^T pair
    tiles [128,2,512]. A is pre-scaled x8, H1 x4, H2pre x16 on device; the
    scales are folded into the (host-scaled) weights / bias adds.
  - attn@V: lhsT = V' pairs [128,2,80h+65], rhs = exp(scores) fp8.

H1 = X@W1 is computed sharded (512 rows/core) and AllGathered in fp8 (2 MB),
as is H2pre. K-bias is dropped (softmax is invariant to per-query shifts).
conv relu+bias evacuations run on DVE (tensor_scalar add+max) with weight
scaling folded on host, keeping ACT free for the 64 softmax EXPs (the ~73 us
scalar-engine floor). Emission interleaves conv matmuls into the attention
j-loop so the PE fills the exp-stall gaps; softmax division uses a 2-step
Newton reciprocal on DVE from a constant seed.
"""

import os
import sys

import numpy as np
import ml_dtypes

try:
    import concourse  # noqa: F401
except ImportError:  # pragma: no cover
    sys.path.insert(0, "/opt/trn_rl_repo")

from concourse import bacc, bass, mybir, tile
from concourse.bass_utils import run_bass_kernel_spmd

P = 128
N_NODES = 4096
E_EDGES = 131072
IN_DIM = 256
HID = 512
CLS = 256
HEADS = 4
HDIM = 64
NC = 8
RPC = N_NODES // NC  # 512 rows per core

BF = mybir.dt.bfloat16
F32 = mybir.dt.float32
F32R = mybir.dt.float32r
F8 = mybir.dt.float8e4
AF = mybir.ActivationFunctionType
ALU = mybir.AluOpType
DR = mybir.MatmulPerfMode.DoubleRow

KCH_IN = IN_DIM // P    # 2
KCH_HID = HID // P      # 4
NT = N_NODES // P       # 32 node tiles
NPAIR = NT // 2         # 16 node-tile pairs
MT_Q = RPC // P         # 4 query tiles per core

# The adjacency factors exactly: A = dinv[dst] * cnt * dinv[src] with integer
# cnt (exact in fp8). dinv scalings ride per-partition evac scales + rank-1
# bias matmuls, so the conv matmuls see NO A-quantization error.
SC_H1 = 4.0      # H1' = 4*dinv*H1 in fp8
SC_H2 = 16.0     # hc = 16*dinv^2*H2pre in fp8
ATTN_FP8 = False  # es/v' in fp8 + DoubleRow attn@V (faster, more error)
SC_V = 8.0 if ATTN_FP8 else 1.0
ES_DT = mybir.dt.float8e4 if ATTN_FP8 else mybir.dt.bfloat16
# Newton seed for 1/D, D = SC_V * sum_k exp(s) ~ SC_V*4096
REC_SEED = 1.0 / (4096.0 * SC_V)

VSLOT = 80  # per-head slot in v' tiles (64 v dims + 1 ones + pad, 16B aligned)


def _emit(tc):
    nc = tc.nc

    # ---------------- I/O ----------------
    xT = nc.dram_tensor("xT", [IN_DIM, N_NODES], BF, kind="ExternalInput")
    xq = nc.dram_tensor("xq", [IN_DIM, RPC], BF, kind="ExternalInput")
    aTd = nc.dram_tensor("aTd", [P, NT, RPC], F8, kind="ExternalInput")
    w1 = nc.dram_tensor("w1", [IN_DIM, HID], BF, kind="ExternalInput")
    w2 = nc.dram_tensor("w2", [HID, HID], BF, kind="ExternalInput")
    lw = nc.dram_tensor("lw", [HID, CLS], BF, kind="ExternalInput")
    ipT = nc.dram_tensor("ipT", [IN_DIM, 3 * IN_DIM], BF, kind="ExternalInput")
    opw = nc.dram_tensor("opw", [IN_DIM, IN_DIM], BF, kind="ExternalInput")
    pw = nc.dram_tensor("pw", [IN_DIM, CLS], BF, kind="ExternalInput")
    bp = nc.dram_tensor("bp", [P, 14], F32, kind="ExternalInput")
    bpd = nc.dram_tensor("bpd", [P, NT], F32, kind="ExternalInput")
    bfrd = nc.dram_tensor("bfrd", [1, 3 * HID], BF, kind="ExternalInput")
    brow = nc.dram_tensor("brow", [1, CLS], BF, kind="ExternalInput")
    out = nc.dram_tensor("out", [RPC, CLS], F32, kind="ExternalOutput")

    from contextlib import ExitStack

    with ExitStack() as ctx:
        sb = ctx.enter_context(tc.tile_pool(name="sb", bufs=1))
        pp = ctx.enter_context(tc.tile_pool(name="pp", bufs=1, space="PSUM"))
        dr = ctx.enter_context(tc.tile_pool(name="dr", bufs=1, space="DRAM"))

        # ---------------- input DMA --------------------------------------
        # H1 needs w1 + both xT halves first; kT/v need ipT next. Split the
        # big early tensors across the two DMA queues; late-consumed tensors
        # (aTs, w2, opw, pw, lw) go after.
        xTs, ipTs = [], []
        for k in range(KCH_IN):
            t = sb.tile([P, N_NODES], BF, name=f"xTs{k}", tag=f"xTs{k}")
            xTs.append(t)
            t2 = sb.tile([P, 3 * IN_DIM], BF, name=f"ipTs{k}", tag=f"ipTs{k}")
            ipTs.append(t2)
        w1s = []
        for k in range(KCH_IN):
            t = sb.tile([P, HID], BF, name=f"w1s{k}", tag=f"w1s{k}")
            nc.gpsimd.dma_start(out=t, in_=w1[k * P:(k + 1) * P, :])
            w1s.append(t)
        nc.sync.dma_start(out=xTs[0], in_=xT[0:P, :])
        nc.gpsimd.dma_start(out=xTs[1], in_=xT[P:2 * P, :])
        nc.sync.dma_start(out=ipTs[0], in_=ipT[0:P, :])
        nc.gpsimd.dma_start(out=ipTs[1], in_=ipT[P:2 * P, :])
        bps = sb.tile([P, 14], F32, name="bps", tag="bps")
        nc.sync.dma_start(out=bps, in_=bp[:, :])
        bpds = sb.tile([P, NT], F32, name="bpds", tag="bpds")
        nc.sync.dma_start(out=bpds, in_=bpd[:, :])
        bfr = sb.tile([1, 3 * HID], BF, name="bfr", tag="bfr")
        nc.sync.dma_start(out=bfr, in_=bfrd[:, :])
        xqs = []
        for k in range(KCH_IN):
            t = sb.tile([P, RPC], BF, name=f"xqs{k}", tag=f"xqs{k}")
            nc.sync.dma_start(out=t, in_=xq[k * P:(k + 1) * P, :])
            xqs.append(t)
        w2s = []
        for k in range(KCH_HID):
            t = sb.tile([P, HID], BF, name=f"w2s{k}", tag=f"w2s{k}")
            nc.gpsimd.dma_start(out=t, in_=w2[k * P:(k + 1) * P, :])
            w2s.append(t)
        opws, pws = [], []
        for k in range(KCH_IN):
            t = sb.tile([P, IN_DIM], BF, name=f"opws{k}", tag=f"opws{k}")
            nc.gpsimd.dma_start(out=t, in_=opw[k * P:(k + 1) * P, :])
            opws.append(t)
            t2 = sb.tile([P, CLS], BF, name=f"pws{k}", tag=f"pws{k}")
            nc.gpsimd.dma_start(out=t2, in_=pw[k * P:(k + 1) * P, :])
            pws.append(t2)
        brows = sb.tile([1, CLS], BF, name="brows", tag="brows")
        nc.gpsimd.dma_start(out=brows, in_=brow[:, :])
        lws = []
        for k in range(KCH_HID):
            t = sb.tile([P, CLS], BF, name=f"lws{k}", tag=f"lws{k}")
            nc.gpsimd.dma_start(out=t, in_=lw[k * P:(k + 1) * P, :])
            lws.append(t)

        ones_bf = sb.tile([1, P], BF, name="ones_bf", tag="ones_bf")
        nc.vector.memset(ones_bf, 1.0)


        # per-partition scale/bias slices
        d4c = [bps[:, m:m + 1] for m in range(0, 4)]          # 4*dinv (m-tile)
        d16c = [bps[:, 4 + m:5 + m] for m in range(0, 4)]     # 16*dinv^2
        dfc = [bps[:, 8 + m:9 + m] for m in range(0, 4)]      # dinv
        bqc = [bps[:, 12 + m:13 + m] for m in range(0, 2)]    # bq

        # big late-consumed input: adjacency counts (conv1 needs it ~30us in)
        aTs = sb.tile([P, NT, RPC], F8, name="aTs", tag="aTs")
        nc.sync.dma_start(out=aTs, in_=aTd[:, :, :])
        # H1' = 4*dinv*(X @ W1) computed REPLICATED (no collective: the first
        # collective would absorb cross-core launch skew as dead PE time).
        # Node-pair fp8 tiles for the conv1 DoubleRow lhsT, produced in the
        # tl0 loop below (all 32 node tiles done by block n=3).
        H1f8 = [sb.tile([P, 2, HID], F8, name=f"H1f8_{j}", tag=f"H1f8_{j}")
                for j in range(NPAIR)]

        def emit_h1_tile(i):
            pt = pp.tile([P, HID], F32, name=f"h1p{i}", tag="mm", bufs=2)
            for k in range(KCH_IN):
                nc.tensor.matmul(out=pt, lhsT=xTs[k][:, i * P:(i + 1) * P],
                                 rhs=w1s[k], start=(k == 0),
                                 stop=(k == KCH_IN - 1))
            # alternate evacuation between DVE and ACT to balance engines
            if i % 2 == 0:
                nc.vector.tensor_scalar_mul(H1f8[i // 2][:, i % 2, :], pt,
                                            bpds[:, i:i + 1])
            else:
                nc.scalar.activation(H1f8[i // 2][:, i % 2, :], pt, AF.Copy,
                                     scale=bpds[:, i:i + 1])

        # ---------------- Phase B: qT (+bq) ---------------------------------
        qTs = []
        for m in range(2):
            pt = pp.tile([P, RPC], F32, name=f"q_ps{m}", tag="mm", bufs=2)
            for k in range(KCH_IN):
                nc.tensor.matmul(out=pt, lhsT=ipTs[k][:, m * P:(m + 1) * P],
                                 rhs=xqs[k], start=(k == 0), stop=(k == KCH_IN - 1))
            t = sb.tile([P, RPC], BF, name=f"qTs{m}", tag=f"qTs{m}")
            nc.vector.tensor_scalar_add(t, pt, bqc[m])
            qTs.append(t)

        # persistent attention state
        kTs = [sb.tile([P, N_NODES], BF, name=f"kTs{m}", tag=f"kTs{m}")
               for m in range(2)]
        vpd = [sb.tile([P, 2, HEADS * VSLOT], ES_DT, name=f"vpd{j}",
                       tag=f"vpd{j}")
               for j in range(NPAIR)]
        # ones columns for the softmax denominator (only the 8 columns the
        # attn@V lhsT slice actually reads; the pad columns stay untouched)
        for j in range(NPAIR):
            nc.gpsimd.memset(
                vpd[j].rearrange("p s (h c) -> p s h c", c=VSLOT)
                [:, :, :, HDIM:HDIM + 1], SC_V)
        oTs = [sb.tile([P, RPC], BF, name=f"oTs{m}", tag=f"oTs{m}")
               for m in range(2)]
        h1Ts, h2Ts = [], []

        def emit_kv_chunk(n):
            # kT columns n*512 .. +512 (both head-pair tiles), no k-bias
            # (softmax is invariant to per-query score shifts).
            for tl in range(2):
                pt = pp.tile([P, 512], F32, name=f"k_ps{tl}_{n}", tag="mm",
                             bufs=2)
                for k in range(KCH_IN):
                    nc.tensor.matmul(
                        out=pt,
                        lhsT=ipTs[k][:, IN_DIM + tl * P:IN_DIM + (tl + 1) * P],
                        rhs=xTs[k][:, n * 512:(n + 1) * 512],
                        start=(k == 0), stop=(k == KCH_IN - 1))
                nc.vector.tensor_copy(out=kTs[tl][:, n * 512:(n + 1) * 512],
                                      in_=pt)
            # v' for node tiles 4n..4n+3 (pairs 2n, 2n+1); the v bias is
            # folded into brow on the host (softmax weights sum to 1, so
            # bv rides through out_proj @ proj as a constant row).
            for i in range(4 * n, 4 * n + 4):
                pt = pp.tile([P, IN_DIM], F32, name=f"v_ps{i}", tag="mm",
                             bufs=2)
                for k in range(KCH_IN):
                    nc.tensor.matmul(out=pt, lhsT=xTs[k][:, i * P:(i + 1) * P],
                                     rhs=ipTs[k][:, 2 * IN_DIM:3 * IN_DIM],
                                     start=(k == 0), stop=(k == KCH_IN - 1))
                vv = (vpd[i // 2][:, i % 2, :]
                      .rearrange("p (h d) -> p h d", h=HEADS)[:, :, 0:HDIM])
                pv = pt.rearrange("p (h d) -> p h d", h=HEADS)
                if SC_V == 1.0:
                    nc.vector.tensor_copy(out=vv, in_=pv)
                else:
                    nc.vector.tensor_scalar_mul(vv, pv, SC_V)

        def emit_attn_j(tl, j, pos):
            # scores for key chunks 2j, 2j+1 (heads 2tl, 2tl+1 packed at
            # partition bases 0/64 -> concurrent PE row groups)
            sss = []
            for hh in range(2):
                s = pp.tile([P, 2, RPC], F32, name=f"sc{tl}_{hh}_{j}",
                            tag=f"sc{hh}", bufs=1)
                sss.append(s)
            for half in range(2):
                i = 2 * j + half
                for hh in range(2):
                    bpart = HDIM * hh
                    nc.tensor.matmul(
                        out=sss[hh][:, half, :],
                        lhsT=kTs[tl][bpart:bpart + HDIM, i * P:(i + 1) * P],
                        rhs=qTs[tl][bpart:bpart + HDIM, :],
                        start=True, stop=True)
            for hh in range(2):
                es = sb.tile([P, 2, RPC], ES_DT, name=f"es{tl}_{hh}_{j}",
                             tag=f"es{hh}", bufs=2)
                nc.scalar.activation(es.rearrange("p a b -> p (a b)"),
                                     sss[hh].rearrange("p a b -> p (a b)"),
                                     AF.Exp, scale=0.125)
                h = 2 * tl + hh
                if ATTN_FP8:
                    nc.tensor.matmul(
                        out=pos[hh],
                        lhsT=vpd[j][:, :, h * VSLOT:h * VSLOT + HDIM + 1],
                        rhs=es,
                        start=(j == 0), stop=(j == NPAIR - 1),
                        perf_mode=DR)
                else:
                    for half in range(2):
                        nc.tensor.matmul(
                            out=pos[hh],
                            lhsT=vpd[j][:, half,
                                        h * VSLOT:h * VSLOT + HDIM + 1],
                            rhs=es[:, half, :],
                            start=(j == 0 and half == 0),
                            stop=(j == NPAIR - 1 and half == 1))

        def emit_division(tl, pos):
            # 1/D via 2-step Newton from constant seed (D ~ 8*4096 +- few %).
            # D rows live at partitions 0 and 32 (engines need quarter-
            # aligned start partitions).
            d2 = sb.tile([33, RPC], F32, name=f"d2_{tl}", tag="d2", bufs=2)
            nc.gpsimd.memset(d2, 1.0 / REC_SEED)
            for hh in range(2):
                nc.vector.tensor_copy(out=d2[32 * hh:32 * hh + 1, :],
                                      in_=pos[hh][HDIM:HDIM + 1, :])
            y1 = sb.tile([33, RPC], F32, name=f"y1_{tl}", tag="y1", bufs=2)
            nc.vector.tensor_scalar(y1, d2, -REC_SEED * REC_SEED,
                                    2.0 * REC_SEED, op0=ALU.mult, op1=ALU.add)
            tt = sb.tile([33, RPC], F32, name=f"tt_{tl}", tag="tt", bufs=2)
            nc.vector.scalar_tensor_tensor(tt, in0=y1, scalar=1.0, in1=d2,
                                           op0=ALU.mult, op1=ALU.mult)
            uu = sb.tile([33, RPC], F32, name=f"uu_{tl}", tag="uu", bufs=2)
            nc.vector.tensor_scalar(uu, tt, -1.0, 2.0, op0=ALU.mult,
                                    op1=ALU.add)
            for hh in range(2):
                # separate base-partition-0 tiles (matmul rhs must align
                # with lhsT's base partition)
                y2 = sb.tile([1, RPC], BF, name=f"y2_{tl}_{hh}", tag="y2",
                             bufs=4)
                nc.vector.tensor_tensor(out=y2,
                                        in0=uu[32 * hh:32 * hh + 1, :],
                                        in1=y1[32 * hh:32 * hh + 1, :],
                                        op=ALU.mult)
                onum = sb.tile([HDIM, RPC], F32, name=f"onum{tl}_{hh}",
                               tag="onum", bufs=2)
                nc.vector.tensor_copy(out=onum, in_=pos[hh][0:HDIM, :])
                pb = pp.tile([HDIM, RPC], F32, name=f"pb{tl}_{hh}", tag="mm",
                             bufs=2)
                nc.tensor.matmul(out=pb, lhsT=ones_bf[0:1, 0:HDIM], rhs=y2,
                                 start=True, stop=True)
                nc.vector.tensor_tensor(
                    out=oTs[tl][HDIM * hh:HDIM * (hh + 1), :],
                    in0=pb, in1=onum, op=ALU.mult)

        def emit_conv1_m(m):
            # psum = sum_src cnt * H1'[src] (+ rank-1 bias 4*b1 (x) 1/dinv)
            pt = pp.tile([P, RPC], F32, name=f"c1p{m}", tag="mm", bufs=2)
            for jj in range(NPAIR):
                nc.tensor.matmul(out=pt,
                                 lhsT=H1f8[jj][:, :, m * P:(m + 1) * P],
                                 rhs=aTs[:, 2 * jj:2 * jj + 2, :],
                                 start=(jj == 0), stop=False,
                                 perf_mode=DR)
            nc.tensor.matmul(out=pt, lhsT=bfr[0:1, m * P:(m + 1) * P],
                             rhs=bfr[0:1, 2 * HID:3 * HID],
                             start=False, stop=True)
            t = sb.tile([P, RPC], BF, name=f"h1T{m}", tag=f"h1T{m}")
            nc.vector.tensor_scalar_max(t, pt, 0.0)
            h1Ts.append(t)

        def emit_conv2_m(m, H2f8):
            # even node-pairs first (AllGather chunk 0), then odd (chunk 1)
            pt = pp.tile([P, RPC], F32, name=f"c2p{m}", tag="mm", bufs=2)
            order = list(range(0, NPAIR, 2)) + list(range(1, NPAIR, 2))
            for idx, jj in enumerate(order):
                nc.tensor.matmul(
                    out=pt,
                    lhsT=H2f8[jj][:, :, m * P:(m + 1) * P],
                    rhs=aTs[:, 2 * jj:2 * jj + 2, :],
                    start=(idx == 0), stop=False,
                    perf_mode=DR)
            nc.tensor.matmul(out=pt, lhsT=bfr[0:1, HID + m * P:HID + (m + 1) * P],
                             rhs=bfr[0:1, 2 * HID:3 * HID],
                             start=False, stop=True)
            t = sb.tile([P, RPC], BF, name=f"h2T{m}", tag=f"h2T{m}")
            nc.vector.tensor_scalar_max(t, pt, 0.0)
            h2Ts.append(t)

        # H2pre AllGather (single collective; conv2 sits at the very end of
        # the PE stream so a slow collective can never head-of-line block
        # the attention matmuls behind it)
        agi_h2 = dr.tile([RPC, HID], F8, name="agi_h2", tag="agi_h2")
        ago_h2 = dr.tile([N_NODES, HID], F8, name="ago_h2", tag="ago_h2",
                         addr_space="Shared")
        H2f8 = [None] * NPAIR

        def emit_h2p_m(m):
            pt = pp.tile([P, HID], F32, name=f"h2p{m}", tag="mm", bufs=2)
            for k in range(KCH_HID):
                nc.tensor.matmul(out=pt, lhsT=h1Ts[k][:, m * P:(m + 1) * P],
                                 rhs=w2s[k], start=(k == 0),
                                 stop=(k == KCH_HID - 1))
            hc = sb.tile([P, HID], F8, name=f"hc{m}", tag="hc", bufs=2)
            nc.vector.tensor_scalar_mul(hc, pt, d16c[m])
            nc.sync.dma_start(out=agi_h2[m * P:(m + 1) * P, :], in_=hc)

        def emit_ag_h2():
            nc.gpsimd.collective_compute(
                "AllGather", ALU.bypass, replica_groups=[list(range(NC))],
                ins=[agi_h2.opt()], outs=[ago_h2.opt()])
            for jj in range(NPAIR):
                t = sb.tile([P, 2, HID], F8, name=f"H2f8_{jj}",
                            tag=f"H2f8_{jj}")
                nc.sync.dma_start(
                    out=t,
                    in_=ago_h2[jj * 2 * P:(jj + 1) * 2 * P, :]
                        .rearrange("(s p) f -> p s f", p=P))
                H2f8[jj] = t

        # ---------------- tl=0 pass: H1/kT/v + attention + conv1 + H2p -----
        # (conv1/H2p have no collective dependence, so interleaving them into
        # the attention stream carries no head-of-line risk)
        pos0 = [pp.tile([HDIM + 1, RPC], F32, name=f"ob0_{hh}", tag="ob",
                        bufs=2) for hh in range(2)]
        for n in range(8):
            emit_kv_chunk(n)
            if n < 2:
                for i in range(16 * n, 16 * n + 16):
                    emit_h1_tile(i)
            for j in (2 * n, 2 * n + 1):
                emit_attn_j(0, j, pos0)
            if 2 <= n < 6:
                emit_conv1_m(n - 2)
            if n == 5:
                for m in range(MT_Q):
                    emit_h2p_m(m)
                emit_ag_h2()
        emit_division(0, pos0)

        # Pc = opw^T @ pw (needed only at the final projection)
        Pcs = []
        for m in range(2):
            pt = pp.tile([P, CLS], F32, name=f"pc_ps{m}", tag="mm", bufs=2)
            for k in range(KCH_IN):
                nc.tensor.matmul(out=pt, lhsT=opws[k][:, m * P:(m + 1) * P],
                                 rhs=pws[k], start=(k == 0), stop=(k == KCH_IN - 1))
            t = sb.tile([P, CLS], BF, name=f"Pcs{m}", tag=f"Pcs{m}")
            nc.vector.tensor_copy(out=t, in_=pt)
            Pcs.append(t)

        pos1 = [pp.tile([HDIM + 1, RPC], F32, name=f"ob1_{hh}", tag="ob",
                        bufs=2) for hh in range(2)]
        for j in range(NPAIR):
            emit_attn_j(1, j, pos1)

        emit_division(1, pos1)

        # ---------------- conv2 last (never blocks attention) --------------
        for m in range(MT_Q):
            emit_conv2_m(m, H2f8)

        xgss = []
        for m in range(MT_Q):
            pg = pp.tile([P, CLS], F32, name=f"xg_ps{m}", tag="mm", bufs=2)
            for k in range(KCH_HID):
                nc.tensor.matmul(out=pg, lhsT=h2Ts[k][:, m * P:(m + 1) * P],
                                 rhs=lws[k], start=(k == 0),
                                 stop=(k == KCH_HID - 1))
            xgs = sb.tile([P, CLS], F32, name=f"xgs{m}", tag=f"xgs{m}")
            nc.vector.tensor_scalar_mul(xgs, pg, dfc[m])
            xgss.append(xgs)

        # ---------------- final: x_gnn + x_proj, relu, store ---------------
        for m in range(MT_Q):
            pj = pp.tile([P, CLS], F32, name=f"xp_ps{m}", tag="mm", bufs=2)
            for k in range(2):
                nc.tensor.matmul(out=pj, lhsT=oTs[k][:, m * P:(m + 1) * P],
                                 rhs=Pcs[k], start=(k == 0), stop=False)
            nc.tensor.matmul(out=pj, lhsT=ones_bf[0:1, 0:P], rhs=brows,
                             start=False, stop=True)
            tadd = sb.tile([P, CLS], F32, name=f"tadd{m}", tag="tadd", bufs=2)
            nc.vector.scalar_tensor_tensor(tadd, in0=pj, scalar=0.0,
                                           in1=xgss[m], op0=ALU.add,
                                           op1=ALU.add)
            osb = sb.tile([P, CLS], F32, name=f"osb{m}", tag="osb", bufs=2)
            nc.vector.tensor_scalar_max(osb, tadd, 0.0)
            nc.sync.dma_start(out=out[m * P:(m + 1) * P, :], in_=osb)


_CACHE = {}


def _get_compiled():
    if "nc" not in _CACHE:
        nc = bacc.Bacc("TRN2", target_bir_lowering=False, debug=False,
                       num_devices=NC)
        with tile.TileContext(nc) as tc:
            _emit(tc)
        nc.compile()
        _CACHE["nc"] = nc
    return _CACHE["nc"]


def _prepare_in_maps(inputs):
    bf16 = ml_dtypes.bfloat16
    fp8 = ml_dtypes.float8_e4m3
    x = np.asarray(inputs["x"], dtype=np.float32)
    ei = np.asarray(inputs["edge_index"]).astype(np.int64)

    loop = np.arange(N_NODES, dtype=np.int64)
    src = np.concatenate([ei[0], loop])
    dst = np.concatenate([ei[1], loop])
    deg = np.bincount(dst, minlength=N_NODES).astype(np.float64)
    dinv = np.where(deg > 0, 1.0 / np.sqrt(deg), 0.0).astype(np.float32)
    # integer edge-count matrix: A = dinv[dst] * cnt * dinv[src], cnt exact
    cnt = np.bincount(dst * N_NODES + src,
                      minlength=N_NODES * N_NODES).astype(np.float32)
    cnt = cnt.reshape(N_NODES, N_NODES)

    xT = np.ascontiguousarray(x.T).astype(bf16)
    w1 = np.asarray(inputs["gcn1_w"], np.float32).astype(bf16)
    # h1T tiles carry (4/dinv)*h1, h2T carry (16/dinv)*h2
    w2 = (np.asarray(inputs["gcn2_w"], np.float32) / SC_H1).astype(bf16)
    lwv = (np.asarray(inputs["lin_w"], np.float32) / SC_H2).astype(bf16)
    ipT = np.ascontiguousarray(
        np.asarray(inputs["in_proj_w"], np.float32).T).astype(bf16)
    opwv = np.asarray(inputs["out_proj_w"], np.float32).astype(bf16)
    pwv = np.asarray(inputs["proj_w"], np.float32).astype(bf16)

    b1 = np.asarray(inputs["gcn1_b"], np.float32) * SC_H1
    b2 = np.asarray(inputs["gcn2_b"], np.float32) * SC_H2
    ipb = np.asarray(inputs["in_proj_b"], np.float32)
    # bv folded through out_proj/proj: softmax weights sum to 1, so the v
    # bias reaches the output as a constant row
    bv = ipb[2 * IN_DIM:3 * IN_DIM]
    opwf = np.asarray(inputs["out_proj_w"], np.float32)
    pwf = np.asarray(inputs["proj_w"], np.float32)
    bprow = (np.asarray(inputs["lin_b"], np.float32)
             + (np.asarray(inputs["out_proj_b"], np.float32) + bv @ opwf.T)
             @ pwf
             + np.asarray(inputs["proj_b"], np.float32))
    browv = np.ascontiguousarray(bprow[None, :]).astype(bf16)

    in_maps = []
    for c in range(NC):
        sl = slice(c * RPC, (c + 1) * RPC)
        dc = dinv[sl]
        bpk = np.zeros((P, 14), np.float32)
        bpk[:, 0:4] = (SC_H1 * dc).reshape(4, P).T
        bpk[:, 4:8] = (SC_H2 * dc * dc).reshape(4, P).T
        bpk[:, 8:12] = dc.reshape(4, P).T
        bpk[:, 12:14] = ipb[0:IN_DIM].reshape(2, P).T
        bpdk = np.ascontiguousarray((SC_H1 * dinv).reshape(NT, P).T)
        bfrk = np.zeros((1, 3 * HID), np.float32)
        bfrk[0, 0:HID] = b1
        bfrk[0, HID:2 * HID] = b2
        bfrk[0, 2 * HID:3 * HID] = 1.0 / dc
        aTc = np.ascontiguousarray(cnt[sl, :].T)  # [src, dst_local]
        aT8 = np.clip(aTc, 0.0, 240.0).astype(fp8)
        aTd = np.ascontiguousarray(
            aT8.reshape(NT, P, RPC).transpose(1, 0, 2))
        in_maps.append({
            "xT": xT,
            "xq": np.ascontiguousarray(xT[:, sl]),
            "aTd": aTd,
            "w1": w1, "w2": w2, "lw": lwv, "ipT": ipT,
            "opw": opwv, "pw": pwv,
            "bp": bpk, "bpd": bpdk, "bfrd": bfrk.astype(bf16),
            "brow": browv,
        })
    return in_maps


def _run(inputs, trace=False):
    nc = _get_compiled()
    in_maps = _prepare_in_maps(inputs)
    res = run_bass_kernel_spmd(nc, in_maps, core_ids=list(range(NC)),
                               trace=trace)
    out = np.concatenate([res.results[c]["out"] for c in range(NC)], axis=0)
    return np.ascontiguousarray(out.astype(np.float32)), res


def kernel(**inputs):
    out, _ = _run(inputs, trace=False)
    return out
